# revision 4
# baseline (speedup 1.0000x reference)
"""Trainium2 Bass kernel for CustomMinkowskiLayerNorm.

Math (matches the jax reference):
    counts[b]  = #points with batch_indices == b           (clamped >= 1)
    mean[b,c]  = sum_{i in b} x[i,c] / counts[b]
    var[b,c]   = sum_{i in b} (x[i,c]-mean)^2 / counts[b]  (= E[x^2]-mean^2)
    out[i,c]   = (x[i,c]-mean[b_i,c]) / sqrt(var[b_i,c]+eps) * gamma[c] + beta[c]

Sharding: batch_indices is sorted and BATCH == n_cores == 8, so each core owns
exactly one batch segment -> all segment reductions are core-local, no
collectives. The host splits at segment boundaries (searchsorted), transposes
each segment to channel-major layout and zero-pads to a fixed shape:

    xt[p, f], p in [0,128): partition p < 64  = channel p,  points [0, F_HALF)
                            partition p >= 64 = channel p-64, points [F_HALF, 2*F_HALF)

Channel-major layout makes the per-channel segment reduction a free-dim
reduction (bn_stats) and the normalization a single per-partition
tensor_scalar (x*s + t) that runs in the DVE 2x fp32 perf mode.

Device program (per core, identical SPMD):
  pass 1: DMA NT tiles of [128, 2048]; 4x bn_stats per tile into a stats
          buffer; the first NCACHE tiles stay resident in SBUF.
  stats:  bn_aggr -> (mean, var) of each padded half-row; convert to raw
          (sum, sumsq); fold partitions p/p+64 and broadcast to both halves
          with one TensorE matmul against a 0/1 fold matrix; apply 1/count
          (host scalar); rstd = 1/sqrt(var+eps) refined with 2 Newton
          iterations (the ACT sqrt table is low-precision); s = gamma*rstd,
          t = beta - mean*s.
  pass 2: x_tile = x_tile * s + t in place (tensor_scalar); DMA back. Only
          the NT-NCACHE non-resident tiles are re-read from HBM.
"""

import os
import sys

for _p in ("/opt/trn_rl_repo", "/root/.axon_site/_ro/trn_rl_repo"):
    if os.path.isdir(_p) and _p not in sys.path:
        sys.path.append(_p)

from contextlib import ExitStack

import numpy as np

import concourse.bacc as bacc
import concourse.tile as tile
from concourse import mybir
from concourse._compat import with_exitstack
from concourse.bass_utils import run_bass_kernel_spmd

F32 = mybir.dt.float32

N = 1_000_000
C = 64
BATCH = 8
EPS = 1e-5

P = 128            # SBUF partitions
F_TILE = 2048      # free elems per tile -> [128, 2048] f32 = 1 MiB per DMA
BN_F = 512         # bn_stats free-dim max
BN_PER_TILE = F_TILE // BN_F   # 4
NCACHE_MAX = 22    # tiles kept resident in SBUF between passes
LOAD_BUFS = 3      # rotating load slots (pass-1 overflow + pass-2 re-reads)

_mult = mybir.AluOpType.mult
_add = mybir.AluOpType.add


def _make_body(f_half: int):
    nt = f_half // F_TILE
    ncache = min(NCACHE_MAX, nt)

    @with_exitstack
    def _body(ctx: ExitStack, tc: tile.TileContext,
              out_ap, xt_ap, invn_ap, gcol_ap, bcol_ap, foldm_ap):
        nc = tc.nc

        cache = ctx.enter_context(tc.tile_pool(name="cache", bufs=ncache))
        lpool = ctx.enter_context(tc.tile_pool(name="lpool", bufs=LOAD_BUFS))
        small = ctx.enter_context(tc.tile_pool(name="small", bufs=1))
        psum = ctx.enter_context(tc.tile_pool(name="psum", bufs=1, space="PSUM"))

        # small per-partition inputs + fold matrix
        invn_sb = small.tile([P, 1], F32, tag="invn")
        gcol_sb = small.tile([P, 1], F32, tag="gcol")
        bcol_sb = small.tile([P, 1], F32, tag="bcol")
        foldm_sb = small.tile([P, P], F32, tag="foldm")
        nc.sync.dma_start(out=invn_sb, in_=invn_ap)
        nc.sync.dma_start(out=gcol_sb, in_=gcol_ap)
        nc.sync.dma_start(out=bcol_sb, in_=bcol_ap)
        nc.sync.dma_start(out=foldm_sb, in_=foldm_ap)

        # Pre-load the ACT sqrt table set while DMA streams pass 1, so the
        # stats chain later doesn't stall on ACT_TABLE_LOAD.
        warm = small.tile([P, 1], F32, tag="warm")
        nc.vector.memset(warm, 1.0)
        nc.scalar.activation(out=warm, in_=warm,
                             func=mybir.ActivationFunctionType.Sqrt)

        stats = small.tile([P, nt * BN_PER_TILE, 6], F32, tag="stats")

        # ---- pass 1: stream all tiles, bn_stats each 512-chunk ----
        cached = []
        for t in range(nt):
            sl = slice(t * F_TILE, (t + 1) * F_TILE)
            if t < ncache:
                xt = cache.tile([P, F_TILE], F32, tag="c")
                cached.append(xt)
            else:
                xt = lpool.tile([P, F_TILE], F32, tag="l")
            nc.sync.dma_start(out=xt, in_=xt_ap[:, sl])
            for j in range(BN_PER_TILE):
                nc.vector.bn_stats(
                    out=stats[:, t * BN_PER_TILE + j, :],
                    in_=xt[:, j * BN_F : (j + 1) * BN_F],
                )

        # ---- aggregate stats ----
        mv = small.tile([P, 2], F32, tag="mv")      # mean/var over padded row
        nc.vector.bn_aggr(out=mv, in_=stats)

        sums = small.tile([P, 2], F32, tag="sums")  # raw (sum, sumsq)
        nc.vector.tensor_scalar_mul(out=sums[:, 0:1], in0=mv[:, 0:1],
                                    scalar1=float(f_half))
        msq = small.tile([P, 1], F32, tag="msq")
        nc.vector.tensor_mul(out=msq, in0=mv[:, 0:1], in1=mv[:, 0:1])
        nc.vector.tensor_add(out=msq, in0=msq, in1=mv[:, 1:2])
        nc.vector.tensor_scalar_mul(out=sums[:, 1:2], in0=msq,
                                    scalar1=float(f_half))

        # ---- fold halves + broadcast: tot[p] = sums[p%64] + sums[p%64+64] ----
        ptot = psum.tile([P, 2], F32, tag="pt")
        nc.tensor.matmul(out=ptot, lhsT=foldm_sb, rhs=sums,
                         start=True, stop=True)
        tot = small.tile([P, 2], F32, tag="tot")
        nc.vector.tensor_copy(out=tot, in_=ptot)

        # ---- per-channel coefficients ----
        mean = small.tile([P, 1], F32, tag="mean")
        nc.vector.tensor_scalar(out=mean, in0=tot[:, 0:1],
                                scalar1=invn_sb[:, 0:1], scalar2=None, op0=_mult)
        var = small.tile([P, 1], F32, tag="var")
        nc.vector.tensor_scalar(out=var, in0=tot[:, 1:2],
                                scalar1=invn_sb[:, 0:1], scalar2=None, op0=_mult)
        m2 = small.tile([P, 1], F32, tag="m2")
        nc.vector.tensor_mul(out=m2, in0=mean, in1=mean)
        nc.vector.tensor_sub(out=var, in0=var, in1=m2)
        # v = max(var, 0) + eps
        v = small.tile([P, 1], F32, tag="v")
        nc.vector.tensor_scalar(out=v, in0=var, scalar1=0.0, scalar2=EPS,
                                op0=mybir.AluOpType.max, op1=_add)
        # r ~= 1/sqrt(v): ACT sqrt + reciprocal, then 2 Newton steps
        r = small.tile([P, 1], F32, tag="r")
        nc.scalar.activation(out=r, in_=v,
                             func=mybir.ActivationFunctionType.Sqrt)
        nc.vector.reciprocal(out=r, in_=r)
        a = small.tile([P, 1], F32, tag="a")
        for _ in range(2):
            nc.vector.tensor_mul(out=a, in0=r, in1=r)
            nc.vector.tensor_mul(out=a, in0=a, in1=v)
            nc.vector.tensor_scalar(out=a, in0=a, scalar1=-0.5, scalar2=1.5,
                                    op0=_mult, op1=_add)
            nc.vector.tensor_mul(out=r, in0=r, in1=a)
        s_col = small.tile([P, 1], F32, tag="s_col")
        nc.vector.tensor_mul(out=s_col, in0=r, in1=gcol_sb)
        t_col = small.tile([P, 1], F32, tag="t_col")
        nc.vector.tensor_mul(out=t_col, in0=mean, in1=s_col)
        nc.vector.tensor_sub(out=t_col, in0=bcol_sb, in1=t_col)

        # ---- pass 2: x = x*s + t, store ----
        # non-resident tiles first: their loads prefetch during the stats chain
        order = list(range(ncache, nt)) + list(range(ncache))
        for t in order:
            sl = slice(t * F_TILE, (t + 1) * F_TILE)
            if t < ncache:
                xt = cached[t]
            else:
                xt = lpool.tile([P, F_TILE], F32, tag="l")
                nc.sync.dma_start(out=xt, in_=xt_ap[:, sl])
            nc.vector.tensor_scalar(out=xt, in0=xt, scalar1=s_col[:, 0:1],
                                    scalar2=t_col[:, 0:1], op0=_mult, op1=_add)
            nc.scalar.dma_start(out=out_ap[:, sl], in_=xt)

    return _body


_NC_CACHE = {}


def _build_program(f_half: int):
    if f_half in _NC_CACHE:
        return _NC_CACHE[f_half]
    nc = bacc.Bacc("TRN2", target_bir_lowering=False, debug=False,
                   num_devices=BATCH)
    xt = nc.dram_tensor("xt", [P, f_half], F32, kind="ExternalInput").ap()
    invn = nc.dram_tensor("invn", [P, 1], F32, kind="ExternalInput").ap()
    gcol = nc.dram_tensor("gcol", [P, 1], F32, kind="ExternalInput").ap()
    bcol = nc.dram_tensor("bcol", [P, 1], F32, kind="ExternalInput").ap()
    foldm = nc.dram_tensor("foldm", [P, P], F32, kind="ExternalInput").ap()
    out = nc.dram_tensor("out", [P, f_half], F32, kind="ExternalOutput").ap()
    with tile.TileContext(nc) as tc:
        _make_body(f_half)(tc, out, xt, invn, gcol, bcol, foldm)
    nc.compile()
    _NC_CACHE[f_half] = nc
    return nc


def _prepare(features, batch_indices, gamma, beta):
    features = np.asarray(features, dtype=np.float32)
    batch_indices = np.asarray(batch_indices, dtype=np.int32)
    gamma = np.asarray(gamma, dtype=np.float32)
    beta = np.asarray(beta, dtype=np.float32)

    bounds = np.searchsorted(batch_indices, np.arange(BATCH + 1), side="left")
    cnts = np.diff(bounds)
    # fixed SPMD shape: half-row length, padded to a multiple of F_TILE
    f_half = max(int(-(-int(cnts.max()) // 2 // F_TILE) * F_TILE), F_TILE)

    gcol = np.concatenate([gamma, gamma]).reshape(P, 1).astype(np.float32)
    bcol = np.concatenate([beta, beta]).reshape(P, 1).astype(np.float32)
    k = np.arange(P)
    foldm = (k[:, None] % C == k[None, :] % C).astype(np.float32)

    in_maps = []
    for b in range(BATCH):
        s, e = int(bounds[b]), int(bounds[b + 1])
        cnt = e - s
        xt = np.zeros((P, f_half), dtype=np.float32)
        n1 = min(cnt, f_half)
        if n1 > 0:
            xt[0:C, :n1] = features[s : s + n1].T
        if cnt > f_half:
            xt[C:P, : cnt - f_half] = features[s + f_half : e].T
        in_maps.append({
            "xt": xt,
            "invn": np.full((P, 1), 1.0 / max(cnt, 1), dtype=np.float32),
            "gcol": gcol,
            "bcol": bcol,
            "foldm": foldm,
        })
    return in_maps, bounds, f_half


def _assemble(results, bounds, f_half):
    out = np.empty((N, C), dtype=np.float32)
    for b in range(BATCH):
        s, e = int(bounds[b]), int(bounds[b + 1])
        cnt = e - s
        if cnt == 0:
            continue
        ot = results[b]["out"]
        n1 = min(cnt, f_half)
        out[s : s + n1] = ot[0:C, :n1].T
        if cnt > f_half:
            out[s + f_half : e] = ot[C:P, : cnt - f_half].T
    return out


def run_with_results(features, batch_indices, gamma, beta, **run_kwargs):
    in_maps, bounds, f_half = _prepare(features, batch_indices, gamma, beta)
    nc = _build_program(f_half)
    res = run_bass_kernel_spmd(nc, in_maps, core_ids=list(range(BATCH)),
                               **run_kwargs)
    return _assemble(res.results, bounds, f_half), res


def kernel(features, batch_indices, gamma, beta):
    out, _ = run_with_results(features, batch_indices, gamma, beta)
    return out


# revision 9
# speedup vs baseline: 1.0842x; 1.0842x over previous
"""Trainium2 Bass kernel for CustomMinkowskiLayerNorm.

Math (matches the jax reference):
    counts[b]  = #points with batch_indices == b           (clamped >= 1)
    mean[b,c]  = sum_{i in b} x[i,c] / counts[b]
    var[b,c]   = sum_{i in b} (x[i,c]-mean)^2 / counts[b]  (= E[x^2]-mean^2)
    out[i,c]   = (x[i,c]-mean[b_i,c]) / sqrt(var[b_i,c]+eps) * gamma[c] + beta[c]

Sharding: batch_indices is sorted and BATCH == n_cores == 8, so each core owns
exactly one batch segment -> all segment reductions are core-local, no
collectives. The host splits at segment boundaries (searchsorted), transposes
each segment to channel-major layout and zero-pads to a fixed shape:

    xt[p, f], p in [0,128): partition p < 64  = channel p,  points [0, F_HALF)
                            partition p >= 64 = channel p-64, points [F_HALF, 2*F_HALF)

Channel-major layout makes the per-channel segment reduction a free-dim
reduction (bn_stats) and the normalization a single per-partition
tensor_scalar (x*s + t) that runs in the DVE 2x fp32 perf mode.

Device program (per core, identical SPMD):
  pass 1: DMA NT tiles of [128, 2048]; 4x bn_stats per tile into a stats
          buffer; the first NCACHE tiles stay resident in SBUF.
  stats:  bn_aggr -> (mean, var) of each padded half-row; convert to raw
          (sum, sumsq); fold partitions p/p+64 and broadcast to both halves
          with one TensorE matmul against a 0/1 fold matrix; apply 1/count
          (host scalar); rstd = 1/sqrt(var+eps) refined with 2 Newton
          iterations (the ACT sqrt table is low-precision); s = gamma*rstd,
          t = beta - mean*s.
  pass 2: x_tile = x_tile * s + t in place (tensor_scalar); DMA back. Only
          the NT-NCACHE non-resident tiles are re-read from HBM.
"""

import os
import sys

for _p in ("/opt/trn_rl_repo", "/root/.axon_site/_ro/trn_rl_repo"):
    if os.path.isdir(_p) and _p not in sys.path:
        sys.path.append(_p)

from contextlib import ExitStack

import numpy as np

import concourse.bacc as bacc
import concourse.tile as tile
from concourse import mybir
from concourse._compat import with_exitstack
from concourse.bass_utils import run_bass_kernel_spmd

F32 = mybir.dt.float32

N = 1_000_000
C = 64
BATCH = 8
EPS = 1e-5

P = 128            # SBUF partitions
F_TILE = 2048      # free elems per tile -> [128, 2048] f32 = 1 MiB per DMA
BN_F = 512         # bn_stats free-dim max
BN_PER_TILE = F_TILE // BN_F   # 4
NCACHE_MAX = 19    # tiles kept resident in SBUF between passes
LOAD_BUFS = 3      # rotating pass-1 load slots
P2_BUFS = 3        # rotating pass-2 re-read slots

_mult = mybir.AluOpType.mult
_add = mybir.AluOpType.add


def _make_body(f_half: int):
    nt = f_half // F_TILE
    ncache = min(NCACHE_MAX, nt)

    @with_exitstack
    def _body(ctx: ExitStack, tc: tile.TileContext,
              out_ap, xt_ap, invn_ap, gcol_ap, bcol_ap, foldm_ap):
        nc = tc.nc

        cache = ctx.enter_context(tc.tile_pool(name="cache", bufs=ncache))
        lpool = ctx.enter_context(tc.tile_pool(name="lpool", bufs=LOAD_BUFS))
        p2pool = ctx.enter_context(tc.tile_pool(name="p2pool", bufs=P2_BUFS))
        small = ctx.enter_context(tc.tile_pool(name="small", bufs=1))
        psum = ctx.enter_context(tc.tile_pool(name="psum", bufs=1, space="PSUM"))

        # small per-partition inputs + fold matrix
        invn_sb = small.tile([P, 1], F32, tag="invn")
        gcol_sb = small.tile([P, 1], F32, tag="gcol")
        bcol_sb = small.tile([P, 1], F32, tag="bcol")
        foldm_sb = small.tile([P, P], F32, tag="foldm")
        nc.sync.dma_start(out=invn_sb, in_=invn_ap)
        nc.sync.dma_start(out=gcol_sb, in_=gcol_ap)
        nc.sync.dma_start(out=bcol_sb, in_=bcol_ap)
        nc.sync.dma_start(out=foldm_sb, in_=foldm_ap)

        # Pre-load the ACT sqrt table set while DMA streams pass 1, so the
        # stats chain later doesn't stall on ACT_TABLE_LOAD.
        warm = small.tile([P, 1], F32, tag="warm")
        nc.vector.memset(warm, 1.0)
        nc.scalar.activation(out=warm, in_=warm,
                             func=mybir.ActivationFunctionType.Sqrt)

        stats = small.tile([P, nt * BN_PER_TILE, 6], F32, tag="stats")

        # ---- pass 1: stream all tiles, bn_stats each 512-chunk ----
        cached = []
        for t in range(nt):
            sl = slice(t * F_TILE, (t + 1) * F_TILE)
            if t < ncache:
                xt = cache.tile([P, F_TILE], F32, tag="c")
                cached.append(xt)
            else:
                xt = lpool.tile([P, F_TILE], F32, tag="l")
            nc.sync.dma_start(out=xt, in_=xt_ap[:, sl])
            for j in range(BN_PER_TILE):
                nc.vector.bn_stats(
                    out=stats[:, t * BN_PER_TILE + j, :],
                    in_=xt[:, j * BN_F : (j + 1) * BN_F],
                )

        # Issue pass-2 re-read DMAs now, on the SWDGE (gpsimd) ring: the sync
        # HWDGE ring is FIFO and its tail loads are DVE-gated, so these would
        # otherwise queue behind them instead of filling idle DMA bandwidth.
        p2tiles = {}
        for t in range(ncache, nt):
            sl = slice(t * F_TILE, (t + 1) * F_TILE)
            xt = p2pool.tile([P, F_TILE], F32, tag="p2")
            nc.gpsimd.dma_start(out=xt, in_=xt_ap[:, sl])
            p2tiles[t] = xt

        # ---- aggregate stats ----
        mv = small.tile([P, 2], F32, tag="mv")      # mean/var over padded row
        nc.vector.bn_aggr(out=mv, in_=stats)

        sums = small.tile([P, 2], F32, tag="sums")  # raw (sum, sumsq)
        nc.vector.tensor_scalar_mul(out=sums[:, 0:1], in0=mv[:, 0:1],
                                    scalar1=float(f_half))
        msq = small.tile([P, 1], F32, tag="msq")
        nc.vector.tensor_mul(out=msq, in0=mv[:, 0:1], in1=mv[:, 0:1])
        nc.vector.tensor_add(out=msq, in0=msq, in1=mv[:, 1:2])
        nc.vector.tensor_scalar_mul(out=sums[:, 1:2], in0=msq,
                                    scalar1=float(f_half))

        # ---- fold halves + broadcast: tot[p] = sums[p%64] + sums[p%64+64] ----
        ptot = psum.tile([P, 2], F32, tag="pt")
        nc.tensor.matmul(out=ptot, lhsT=foldm_sb, rhs=sums,
                         start=True, stop=True)
        tot = small.tile([P, 2], F32, tag="tot")
        nc.vector.tensor_copy(out=tot, in_=ptot)

        # ---- per-channel coefficients ----
        mean = small.tile([P, 1], F32, tag="mean")
        nc.vector.tensor_scalar(out=mean, in0=tot[:, 0:1],
                                scalar1=invn_sb[:, 0:1], scalar2=None, op0=_mult)
        var = small.tile([P, 1], F32, tag="var")
        nc.vector.tensor_scalar(out=var, in0=tot[:, 1:2],
                                scalar1=invn_sb[:, 0:1], scalar2=None, op0=_mult)
        m2 = small.tile([P, 1], F32, tag="m2")
        nc.vector.tensor_mul(out=m2, in0=mean, in1=mean)
        nc.vector.tensor_sub(out=var, in0=var, in1=m2)
        # v = max(var, 0) + eps
        v = small.tile([P, 1], F32, tag="v")
        nc.vector.tensor_scalar(out=v, in0=var, scalar1=0.0, scalar2=EPS,
                                op0=mybir.AluOpType.max, op1=_add)
        # r ~= 1/sqrt(v): ACT sqrt + reciprocal, then 2 Newton steps
        r = small.tile([P, 1], F32, tag="r")
        nc.scalar.activation(out=r, in_=v,
                             func=mybir.ActivationFunctionType.Sqrt)
        nc.vector.reciprocal(out=r, in_=r)
        a = small.tile([P, 1], F32, tag="a")
        for _ in range(2):
            nc.vector.tensor_mul(out=a, in0=r, in1=r)
            nc.vector.tensor_mul(out=a, in0=a, in1=v)
            nc.vector.tensor_scalar(out=a, in0=a, scalar1=-0.5, scalar2=1.5,
                                    op0=_mult, op1=_add)
            nc.vector.tensor_mul(out=r, in0=r, in1=a)
        s_col = small.tile([P, 1], F32, tag="s_col")
        nc.vector.tensor_mul(out=s_col, in0=r, in1=gcol_sb)
        t_col = small.tile([P, 1], F32, tag="t_col")
        nc.vector.tensor_mul(out=t_col, in0=mean, in1=s_col)
        nc.vector.tensor_sub(out=t_col, in0=bcol_sb, in1=t_col)

        # ---- pass 2: x = x*s + t, store ----
        # interleave non-resident tiles among resident ones so their re-read
        # slots recycle while stores stream
        cu, uu = list(range(ncache)), list(range(ncache, nt))
        order = []
        while cu or uu:
            if uu:
                order.append(uu.pop(0))
            order.extend(cu[:2])
            del cu[:2]
        for t in order:
            sl = slice(t * F_TILE, (t + 1) * F_TILE)
            xt = cached[t] if t < ncache else p2tiles[t]
            nc.vector.tensor_scalar(out=xt, in0=xt, scalar1=s_col[:, 0:1],
                                    scalar2=t_col[:, 0:1], op0=_mult, op1=_add)
            nc.scalar.dma_start(out=out_ap[:, sl], in_=xt)

    return _body


_NC_CACHE = {}


def _build_program(f_half: int):
    if f_half in _NC_CACHE:
        return _NC_CACHE[f_half]
    nc = bacc.Bacc("TRN2", target_bir_lowering=False, debug=False,
                   num_devices=BATCH)
    xt = nc.dram_tensor("xt", [P, f_half], F32, kind="ExternalInput").ap()
    invn = nc.dram_tensor("invn", [P, 1], F32, kind="ExternalInput").ap()
    gcol = nc.dram_tensor("gcol", [P, 1], F32, kind="ExternalInput").ap()
    bcol = nc.dram_tensor("bcol", [P, 1], F32, kind="ExternalInput").ap()
    foldm = nc.dram_tensor("foldm", [P, P], F32, kind="ExternalInput").ap()
    out = nc.dram_tensor("out", [P, f_half], F32, kind="ExternalOutput").ap()
    with tile.TileContext(nc) as tc:
        _make_body(f_half)(tc, out, xt, invn, gcol, bcol, foldm)
    nc.compile()
    _NC_CACHE[f_half] = nc
    return nc


def _prepare(features, batch_indices, gamma, beta):
    features = np.asarray(features, dtype=np.float32)
    batch_indices = np.asarray(batch_indices, dtype=np.int32)
    gamma = np.asarray(gamma, dtype=np.float32)
    beta = np.asarray(beta, dtype=np.float32)

    bounds = np.searchsorted(batch_indices, np.arange(BATCH + 1), side="left")
    cnts = np.diff(bounds)
    # fixed SPMD shape: half-row length, padded to a multiple of F_TILE
    f_half = max(int(-(-int(cnts.max()) // 2 // F_TILE) * F_TILE), F_TILE)

    gcol = np.concatenate([gamma, gamma]).reshape(P, 1).astype(np.float32)
    bcol = np.concatenate([beta, beta]).reshape(P, 1).astype(np.float32)
    k = np.arange(P)
    foldm = (k[:, None] % C == k[None, :] % C).astype(np.float32)

    in_maps = []
    for b in range(BATCH):
        s, e = int(bounds[b]), int(bounds[b + 1])
        cnt = e - s
        xt = np.zeros((P, f_half), dtype=np.float32)
        n1 = min(cnt, f_half)
        if n1 > 0:
            xt[0:C, :n1] = features[s : s + n1].T
        if cnt > f_half:
            xt[C:P, : cnt - f_half] = features[s + f_half : e].T
        in_maps.append({
            "xt": xt,
            "invn": np.full((P, 1), 1.0 / max(cnt, 1), dtype=np.float32),
            "gcol": gcol,
            "bcol": bcol,
            "foldm": foldm,
        })
    return in_maps, bounds, f_half


def _assemble(results, bounds, f_half):
    out = np.empty((N, C), dtype=np.float32)
    for b in range(BATCH):
        s, e = int(bounds[b]), int(bounds[b + 1])
        cnt = e - s
        if cnt == 0:
            continue
        ot = results[b]["out"]
        n1 = min(cnt, f_half)
        out[s : s + n1] = ot[0:C, :n1].T
        if cnt > f_half:
            out[s + f_half : e] = ot[C:P, : cnt - f_half].T
    return out


def run_with_results(features, batch_indices, gamma, beta, **run_kwargs):
    in_maps, bounds, f_half = _prepare(features, batch_indices, gamma, beta)
    nc = _build_program(f_half)
    res = run_bass_kernel_spmd(nc, in_maps, core_ids=list(range(BATCH)),
                               **run_kwargs)
    return _assemble(res.results, bounds, f_half), res


def kernel(features, batch_indices, gamma, beta):
    out, _ = run_with_results(features, batch_indices, gamma, beta)
    return out
